# revision 14
# baseline (speedup 1.0000x reference)
"""Canny edge detection (nn_CannyEdge_83330955477492) on 8 Trainium2 cores.

Pipeline reproduced from the reference:
  - The reference's "gaussian blur" (sigma=0.05, and a 2x2 kernel thanks to
    arange(-(3//2)+1, 3//2+1) == [0,1]) is exactly a top-left crop of the
    reflect-padded image: blur[i,j] = x[R(i-1), R(j-1)], R(-1)=1 -> 1025x1025.
  - Sobel gx/gy on the reflect-padded blur (correlation).
  - Direction binning: done with exact slope comparisons against
    tan(22.5deg)/tan(67.5deg) instead of atan2 (bit-identical except for
    pixels within 1 ulp of a bin boundary).
  - Magnitude comparisons use gx^2+gy^2 (monotone equivalent of sqrt).
  - NMS via shifted maxes per bin; thresholds at 50^2/100^2.

Sharding: pure data parallel, 2 images per core.

Device layout: one strip of 122 partitions (61 per image, 17 output rows
per partition), column chunks of ~79. All (dy,dx) stencil shifts are
free-dim AP offsets on flattened (row, col) per-partition patches. The host
pre-builds a padded input Q (reflect/dup rows+cols baked in) so every DMA
is a single regular access pattern; outputs go to row-padded DRAM tensors
the host crops.
"""
import numpy as np

# ---------------------------------------------------------------- geometry
NIMG = 2            # images per core
H = 1024            # input image size
HO = 1025           # output size (blur is 1025x1025)
RPP = 17            # output rows per partition
PPI = 61            # partitions per image (61*17 = 1037 >= 1025)
NPART = NIMG * PPI  # 122
QROWS_PER_IMG = RPP * PPI        # 1037 (input Q image stride, rows)
QROWS = NIMG * QROWS_PER_IMG + 4  # 2078 (tail pad for last partition's 21-row window)
QCOLS = 1032        # 1 (zero "P col -1") + 1027 P cols + 4 pad
OROWS_PER_IMG = RPP * PPI        # 1037
OROWS = NIMG * OROWS_PER_IMG     # 2074
OCOLS = HO          # 1025

# column chunking: widths summing to 1025
CHUNKS = [103] * 5 + [102] * 5
assert sum(CHUNKS) == HO
WBMAX = max(CHUNKS) + 2

_T1 = float(np.float32(np.tan(np.deg2rad(22.5))))
_T2 = float(np.float32(np.tan(np.deg2rad(67.5))))
MIN2 = float(np.float32(50.0 * 50.0))
MAX2 = float(np.float32(100.0 * 100.0))

_NC = None
LAST_RESULTS = None  # stashed BassKernelResults for test.py


# ------------------------------------------------- walrus 1-wait workaround
def _split_multiwaits(nc):
    """This walrus build rejects >1 sync wait per instruction: move extra
    waits onto fresh same-engine NOPs inserted right before the carrier."""
    import concourse.mybir as mybir

    n_split = 0
    for fn in nc.m.functions:
        for bb in fn.blocks:
            insts = list(bb.instructions)
            if not any(
                i.sync_info is not None
                and i.sync_info.on_wait
                and len(i.sync_info.on_wait) > 1
                for i in insts
            ):
                continue
            out = []
            for inst in insts:
                si = inst.sync_info
                if si is not None and si.on_wait and len(si.on_wait) > 1:
                    waits = list(si.on_wait)
                    eng = nc.engines[inst.engine]
                    for w in waits[:-1]:
                        nop = eng.nop(hint="waitsplit")
                        # eng.nop() appended to nc.cur_bb — remove it there
                        # (it must live ONLY at its split position, else the
                        # duplicate runs after sem cleanup and deadlocks).
                        host = nc.cur_bb.bb
                        lst = list(host.instructions)
                        assert lst and lst[-1].name == nop.ins.name
                        _set_insts(host, lst[:-1])
                        nop.ins.sync_info = mybir.SyncInfo(
                            on_wait=[w], on_update=[]
                        )
                        out.append(nop.ins)
                        n_split += 1
                    si.on_wait = waits[-1:]
                out.append(inst)
            _set_insts(bb, out)
    return n_split


def _set_insts(bb, lst):
    try:
        bb.instructions = lst
    except Exception:
        bb.instructions.clear()
        bb.instructions.extend(lst)


def _flat(t):
    """[P, a, b] tile -> [P, a*b] AP (free dims are contiguous in SBUF)."""
    return t[:].rearrange("p a b -> p (a b)")


# ------------------------------------------------------------ device build
def _build_nc():
    import concourse.bass as bass
    import concourse.tile as tile
    import concourse.mybir as mybir
    from concourse.ap import AP

    f32 = mybir.dt.float32
    f16 = mybir.dt.float16
    Alu = mybir.AluOpType
    Act = mybir.ActivationFunctionType

    nc = bass.Bass("TRN2", target_bir_lowering=False, debug=False, num_devices=8)
    qin = nc.declare_dram_parameter("qin", [QROWS, QCOLS], f32, isOutput=False)
    bmask = nc.declare_dram_parameter("bmask", [NPART, 19, WBMAX], f32,
                                      isOutput=False)
    o_img = nc.declare_dram_parameter("o_img", [OROWS, OCOLS], f32, isOutput=True)
    o_week = nc.declare_dram_parameter("o_week", [OROWS, OCOLS], f32, isOutput=True)
    o_sure = nc.declare_dram_parameter("o_sure", [OROWS, OCOLS], f32, isOutput=True)
    outs = {"o_img": o_img, "o_week": o_week, "o_sure": o_sure}

    with tile.TileContext(nc) as tc:
        with (
            tc.tile_pool(name="io2", bufs=2) as io2,    # load/store overlap
            tc.tile_pool(name="mid", bufs=1) as mid,    # per-chunk intermediates
            tc.tile_pool(name="cst", bufs=1) as cst,    # persistent constants
        ):
            # border-row mask: zeros at ang rows outside the image
            # (compute APs can't start mid-quadrant, so memsets on partitions
            # 60/61/121 are rejected by the verifier -> mask multiply instead)
            bm = cst.tile([NPART, 19, WBMAX], f32, tag="bm")
            nc.sync.dma_start(out=bm[:], in_=bmask[:])
            a = 0  # output column offset of this chunk
            for ci, cw in enumerate(CHUNKS):
                first = ci == 0
                last = ci == len(CHUNKS) - 1
                WA = cw + 4   # loaded cols
                WB = cw + 2   # ang cols
                # ---- load: partition p gets Q rows 17p..17p+20, cols a..a+WA
                tin = io2.tile([NPART, 21, WA], f32, tag="tin")
                src = AP(qin, a, [[RPP * QCOLS, NPART], [QCOLS, 21], [1, WA]])
                nc.sync.dma_start(out=tin[:], in_=src)

                # ---- row stencils (all 21 rows, WB cols)
                tt = mid.tile([NPART, 21, WB], f32, tag="tt")
                nc.vector.tensor_tensor(
                    out=tt[:], in0=tin[:, :, 0:WB], in1=tin[:, :, 2:WA],
                    op=Alu.add)
                rsm = mid.tile([NPART, 21, WB], f32, tag="rsm")
                nc.vector.scalar_tensor_tensor(
                    out=rsm[:], in0=tin[:, :, 1:WB + 1], scalar=2.0,
                    in1=tt[:], op0=Alu.mult, op1=Alu.add)
                dd = mid.tile([NPART, 21, WB], f32, tag="dd")
                nc.vector.tensor_tensor(
                    out=dd[:], in0=tin[:, :, 2:WA], in1=tin[:, :, 0:WB],
                    op=Alu.subtract)

                # ---- vertical stencils (19 rows): gx, gy
                t2 = mid.tile([NPART, 19, WB], f32, tag="t2")
                nc.vector.tensor_tensor(
                    out=t2[:], in0=dd[:, 0:19, :], in1=dd[:, 2:21, :],
                    op=Alu.add)
                gx = mid.tile([NPART, 19, WB], f32, tag="gx")
                nc.vector.scalar_tensor_tensor(
                    out=gx[:], in0=dd[:, 1:20, :], scalar=2.0, in1=t2[:],
                    op0=Alu.mult, op1=Alu.add)
                gy = mid.tile([NPART, 19, WB], f32, tag="gy")
                nc.vector.tensor_tensor(
                    out=gy[:], in0=rsm[:, 2:21, :], in1=rsm[:, 0:19, :],
                    op=Alu.subtract)

                # ---- sign, abs (ACT), squares (ACT, exact), magnitude^2
                # slot reuse: sg->t2's slot, ax->tt's, ay->dd's (all dead)
                sg = mid.tile([NPART, 19, WB], f32, tag="t2")
                nc.vector.tensor_tensor(out=sg[:], in0=gx[:], in1=gy[:],
                                        op=Alu.mult)
                ax = mid.tile([NPART, 19, WB], f32, tag="tt")
                nc.scalar.activation(out=ax[:], in_=gx[:], func=Act.Abs)
                ay = mid.tile([NPART, 19, WB], f32, tag="dd")
                nc.scalar.activation(out=ay[:], in_=gy[:], func=Act.Abs)
                c0 = mid.tile([NPART, 19, WB], f32, tag="c0")
                nc.vector.scalar_tensor_tensor(
                    out=c0[:], in0=ax[:], scalar=_T1, in1=ay[:],
                    op0=Alu.mult, op1=Alu.is_ge)
                d2 = mid.tile([NPART, 19, WB], f32, tag="d2")
                nc.vector.scalar_tensor_tensor(
                    out=d2[:], in0=ax[:], scalar=_T2, in1=ay[:],
                    op0=Alu.mult, op1=Alu.is_gt)
                # squares on ACT (verified bit-exact vs IEEE mult on HW)
                nc.scalar.activation(out=gx[:], in_=gx[:], func=Act.Square)
                nc.scalar.activation(out=gy[:], in_=gy[:], func=Act.Square)
                mm = mid.tile([NPART, 19, WB], f32, tag="mm")
                nc.vector.tensor_tensor(out=mm[:], in0=gx[:], in1=gy[:],
                                        op=Alu.add)

                # ---- zero magnitude outside the image (NMS zero-padding)
                nc.vector.tensor_tensor(out=mm[:], in0=mm[:],
                                        in1=bm[:, :, 0:WB], op=Alu.mult)
                if first:
                    nc.gpsimd.memset(mm[:, :, 0:1], 0.0)      # ang col -1
                if last:
                    nc.gpsimd.memset(mm[:, :, WB - 1:WB], 0.0)  # ang col 1025

                # ---- masked magnitudes; bins processed 0,2,3,1 so ang0/
                # ang2/ang3 can share ONE slot (angA); ang1 lands in md2.
                img50 = mid.tile([NPART, 17, cw], f16, tag="img50")
                sure = mid.tile([NPART, 17, cw], f16, tag="sure")
                p16 = mid.tile([NPART, 17, cw], f16, tag="p16")

                def nms_bin(ang, r1, c1, r2, c2, acc):
                    qt = mid.tile([NPART, 17, cw], f32, tag="rsm")
                    nc.vector.tensor_tensor(
                        out=qt[:],
                        in0=ang[:, r1:r1 + 17, c1:c1 + cw],
                        in1=ang[:, r2:r2 + 17, c2:c2 + cw],
                        op=Alu.max)
                    cen = ang[:, 1:18, 1:1 + cw]
                    for thr, dstn in ((MIN2, img50), (MAX2, sure)):
                        dst = dstn if not acc else p16
                        nc.vector.scalar_tensor_tensor(
                            out=dst[:], in0=qt[:], scalar=thr, in1=cen,
                            op0=Alu.max, op1=Alu.is_le)
                        if acc:
                            # flat contiguous f16 APs -> DVE 2x mode
                            nc.vector.tensor_tensor(
                                out=_flat(dstn), in0=_flat(dstn),
                                in1=_flat(p16), op=Alu.add)

                md2 = mid.tile([NPART, 19, WB], f32, tag="md2")
                nc.vector.tensor_tensor(out=md2[:], in0=mm[:], in1=d2[:],
                                        op=Alu.mult)      # Md2 = M*d2
                angA = mid.tile([NPART, 19, WB], f32, tag="angA")
                nc.vector.tensor_tensor(out=angA[:], in0=mm[:], in1=md2[:],
                                        op=Alu.subtract)  # ang2 = M - Md2
                nms_bin(angA, 0, 1, 2, 1, acc=False)      # bin2: up/down
                angB = mid.tile([NPART, 19, WB], f32, tag="angA")
                nc.vector.tensor_tensor(out=angB[:], in0=md2[:], in1=c0[:],
                                        op=Alu.mult)      # ang0 = Md2*c0
                nms_bin(angB, 1, 0, 1, 2, acc=True)       # bin0: left/right
                # mdiag = Md2 - ang0 (in place; angB=ang0 still live)
                nc.vector.tensor_tensor(out=md2[:], in0=md2[:], in1=angB[:],
                                        op=Alu.subtract)
                angC = mid.tile([NPART, 19, WB], f32, tag="angA")
                nc.vector.scalar_tensor_tensor(
                    out=angC[:], in0=sg[:], scalar=0.0, in1=md2[:],
                    op0=Alu.is_gt, op1=Alu.mult)          # ang3 = (s>0)*mdiag
                nms_bin(angC, 0, 0, 2, 2, acc=True)       # bin3: main diag
                # ang1 = mdiag - ang3 (in place on md2)
                nc.vector.tensor_tensor(out=md2[:], in0=md2[:], in1=angC[:],
                                        op=Alu.subtract)
                nms_bin(md2, 0, 2, 2, 0, acc=True)        # bin1: anti diag
                # weak = img50 - sure (values in {0,1})
                nc.vector.tensor_tensor(out=_flat(p16), in0=_flat(img50),
                                        in1=_flat(sure), op=Alu.subtract)

                # ---- scale to outputs on ACT (exact: inputs are 0/1)
                e_img = io2.tile([NPART, 17, cw], f32, tag="e_img")
                nc.scalar.activation(out=e_img[:], in_=img50[:],
                                     func=Act.Copy, scale=255.5)
                e_week = io2.tile([NPART, 17, cw], f32, tag="e_week")
                nc.scalar.activation(out=e_week[:], in_=p16[:],
                                     func=Act.Copy, scale=255.0)
                e_sure = io2.tile([NPART, 17, cw], f32, tag="e_sure")
                nc.scalar.activation(out=e_sure[:], in_=sure[:],
                                     func=Act.Copy, scale=255.0)

                for t, name in ((e_img, "o_img"), (e_week, "o_week"),
                                (e_sure, "o_sure")):
                    dst = AP(outs[name], a,
                             [[RPP * OCOLS, NPART], [OCOLS, RPP], [1, cw]])
                    nc.sync.dma_start(out=dst, in_=t[:])
                a += cw

    _split_multiwaits(nc)
    return nc


def _get_nc():
    global _NC
    if _NC is None:
        _NC = _build_nc()
    return _NC


# ------------------------------------------------------------- host helpers
def _build_q(images):
    """images: (16, 1024, 1024) f32 -> per-core padded Q (8, QROWS, QCOLS).

    Q[img_block] row r, col c = P[r-1, c-1] where P is the twice-padded
    image: P index list (both dims) = [0, 1, 0, 1, 2, ..., 1023, 1022].
    """
    idx = np.empty(1027, np.int64)
    idx[0] = 0
    idx[1] = 1
    idx[2:1026] = np.arange(1024)
    idx[1026] = 1022
    qs = np.zeros((8, QROWS, QCOLS), np.float32)
    for core in range(8):
        for k in range(NIMG):
            im = images[core * NIMG + k]
            p = im[idx][:, idx]  # (1027, 1027)
            base = k * QROWS_PER_IMG
            qs[core, base + 1: base + 1028, 1:1028] = p
    return qs


def kernel(images):
    global LAST_RESULTS
    from concourse.bass_utils import run_bass_kernel_spmd

    images = np.asarray(images, dtype=np.float32)
    assert images.shape == (16, 1024, 1024, 1), images.shape
    qs = _build_q(images[:, :, :, 0])

    bm = np.ones((NPART, 19, WBMAX), np.float32)
    for base in (0, PPI):
        bm[base, 0, :] = 0.0          # ang row -1 of each image
        bm[base + PPI - 1, 6:, :] = 0.0  # ang rows >= 1025 of each image

    nc = _get_nc()
    in_maps = [{"qin": qs[c], "bmask": bm} for c in range(8)]
    res = run_bass_kernel_spmd(nc, in_maps, list(range(8)))
    LAST_RESULTS = res

    out = []
    for name in ("o_img", "o_week", "o_sure"):
        full = np.empty((16, HO, HO, 1), np.float32)
        for c in range(8):
            r = res.results[c][name].reshape(NIMG, OROWS_PER_IMG, OCOLS)
            full[c * NIMG: c * NIMG + NIMG, :, :, 0] = r[:, :HO, :]
        out.append(full)
    return tuple(out)


# revision 18
# speedup vs baseline: 1.1445x; 1.1445x over previous
"""Canny edge detection (nn_CannyEdge_83330955477492) on 8 Trainium2 cores.

Pipeline reproduced from the reference:
  - The reference's "gaussian blur" (sigma=0.05, and a 2x2 kernel thanks to
    arange(-(3//2)+1, 3//2+1) == [0,1]) is exactly a top-left crop of the
    reflect-padded image: blur[i,j] = x[R(i-1), R(j-1)], R(-1)=1 -> 1025x1025.
  - Sobel gx/gy on the reflect-padded blur (correlation).
  - Direction binning: done with exact slope comparisons against
    tan(22.5deg)/tan(67.5deg) instead of atan2 (bit-identical except for
    pixels within 1 ulp of a bin boundary).
  - Magnitude comparisons use gx^2+gy^2 (monotone equivalent of sqrt).
  - NMS via shifted maxes per bin; thresholds at 50^2/100^2.

Sharding: pure data parallel, 2 images per core.

Device layout: one strip of 122 partitions (61 per image, 17 output rows
per partition), column chunks of 103/102. All (dy,dx) stencil shifts are
free-dim AP offsets on flattened (row, col) per-partition patches. The host
pre-builds a padded input Q (reflect/dup rows+cols baked in) so every DMA
is a single regular access pattern; outputs go to row-padded DRAM tensors
the host crops.
"""
import numpy as np

# ---------------------------------------------------------------- geometry
NIMG = 2            # images per core
H = 1024            # input image size
HO = 1025           # output size (blur is 1025x1025)
RPP = 17            # output rows per partition
PPI = 61            # partitions per image (61*17 = 1037 >= 1025)
NPART = NIMG * PPI  # 122
QROWS_PER_IMG = RPP * PPI        # 1037 (input Q image stride, rows)
QROWS = NIMG * QROWS_PER_IMG + 4  # 2078 (tail pad for last partition's 21-row window)
QCOLS = 1032        # 1 (zero "P col -1") + 1027 P cols + 4 pad
OROWS_PER_IMG = RPP * PPI        # 1037
OROWS = NIMG * OROWS_PER_IMG     # 2074
OCOLS = HO          # 1025

# column chunking: widths summing to 1025
CHUNKS = [103] * 5 + [102] * 5
assert sum(CHUNKS) == HO
WBMAX = max(CHUNKS) + 2

_T1 = float(np.float32(np.tan(np.deg2rad(22.5))))
_T2 = float(np.float32(np.tan(np.deg2rad(67.5))))
MIN2 = float(np.float32(50.0 * 50.0))
MAX2 = float(np.float32(100.0 * 100.0))

_NC = None
LAST_RESULTS = None  # stashed BassKernelResults for test.py


# ------------------------------------------------- walrus 1-wait workaround
def _split_multiwaits(nc):
    """This walrus build rejects >1 sync wait per instruction: move extra
    waits onto fresh same-engine NOPs inserted right before the carrier."""
    import concourse.mybir as mybir

    n_split = 0
    for fn in nc.m.functions:
        for bb in fn.blocks:
            insts = list(bb.instructions)
            if not any(
                i.sync_info is not None
                and i.sync_info.on_wait
                and len(i.sync_info.on_wait) > 1
                for i in insts
            ):
                continue
            out = []
            for inst in insts:
                si = inst.sync_info
                if si is not None and si.on_wait and len(si.on_wait) > 1:
                    waits = list(si.on_wait)
                    eng = nc.engines[inst.engine]
                    for w in waits[:-1]:
                        nop = eng.nop(hint="waitsplit")
                        # eng.nop() appended to nc.cur_bb — remove it there
                        # (it must live ONLY at its split position, else the
                        # duplicate runs after sem cleanup and deadlocks).
                        host = nc.cur_bb.bb
                        lst = list(host.instructions)
                        assert lst and lst[-1].name == nop.ins.name
                        _set_insts(host, lst[:-1])
                        nop.ins.sync_info = mybir.SyncInfo(
                            on_wait=[w], on_update=[]
                        )
                        out.append(nop.ins)
                        n_split += 1
                    si.on_wait = waits[-1:]
                out.append(inst)
            _set_insts(bb, out)
    return n_split


def _set_insts(bb, lst):
    try:
        bb.instructions = lst
    except Exception:
        bb.instructions.clear()
        bb.instructions.extend(lst)


def _flat(t):
    """[P, a, b] tile -> [P, a*b] AP (free dims are contiguous in SBUF)."""
    return t[:].rearrange("p a b -> p (a b)")


# ------------------------------------------------------------ device build
def _build_nc():
    import concourse.bass as bass
    import concourse.tile as tile
    import concourse.mybir as mybir
    from concourse.ap import AP

    f32 = mybir.dt.float32
    f16 = mybir.dt.float16
    Alu = mybir.AluOpType
    Act = mybir.ActivationFunctionType

    nc = bass.Bass("TRN2", target_bir_lowering=False, debug=False, num_devices=8)
    qin = nc.declare_dram_parameter("qin", [QROWS, QCOLS], f32, isOutput=False)
    bmask = nc.declare_dram_parameter("bmask", [NPART, 19, WBMAX], f32,
                                      isOutput=False)
    o_img = nc.declare_dram_parameter("o_img", [OROWS, OCOLS], f32, isOutput=True)
    o_week = nc.declare_dram_parameter("o_week", [OROWS, OCOLS], f32, isOutput=True)
    o_sure = nc.declare_dram_parameter("o_sure", [OROWS, OCOLS], f32, isOutput=True)
    outs = {"o_img": o_img, "o_week": o_week, "o_sure": o_sure}

    with tile.TileContext(nc) as tc:
        with (
            tc.tile_pool(name="io2", bufs=2) as io2,    # load/store overlap
            tc.tile_pool(name="mid", bufs=1) as mid,    # per-chunk intermediates
            tc.tile_pool(name="cst", bufs=1) as cst,    # persistent constants
        ):
            # border-row mask: zeros at ang rows outside the image
            # (compute APs can't start mid-quadrant, so memsets on partitions
            # 60/61/121 are rejected by the verifier -> mask multiply instead)
            bm = cst.tile([NPART, 19, WBMAX], f32, tag="bm")
            nc.sync.dma_start(out=bm[:], in_=bmask[:])
            a = 0  # output column offset of this chunk
            for ci, cw in enumerate(CHUNKS):
                first = ci == 0
                last = ci == len(CHUNKS) - 1
                WA = cw + 4   # loaded cols
                WB = cw + 2   # ang cols
                # ---- load: partition p gets Q rows 17p..17p+20, cols a..a+WA
                tin = io2.tile([NPART, 21, WA], f32, tag="tin")
                src = AP(qin, a, [[RPP * QCOLS, NPART], [QCOLS, 21], [1, WA]])
                nc.sync.dma_start(out=tin[:], in_=src)

                # ---- row stencils (all 21 rows, WB cols)
                tt = mid.tile([NPART, 21, WB], f32, tag="tt")
                nc.vector.tensor_tensor(
                    out=tt[:], in0=tin[:, :, 0:WB], in1=tin[:, :, 2:WA],
                    op=Alu.add)
                rsm = mid.tile([NPART, 21, WB], f32, tag="rsm")
                nc.vector.scalar_tensor_tensor(
                    out=rsm[:], in0=tin[:, :, 1:WB + 1], scalar=2.0,
                    in1=tt[:], op0=Alu.mult, op1=Alu.add)
                dd = mid.tile([NPART, 21, WB], f32, tag="dd")
                nc.vector.tensor_tensor(
                    out=dd[:], in0=tin[:, :, 2:WA], in1=tin[:, :, 0:WB],
                    op=Alu.subtract)

                # ---- vertical stencils (19 rows): gx, gy
                t2 = mid.tile([NPART, 19, WB], f32, tag="t2")
                nc.vector.tensor_tensor(
                    out=t2[:], in0=dd[:, 0:19, :], in1=dd[:, 2:21, :],
                    op=Alu.add)
                gx = mid.tile([NPART, 19, WB], f32, tag="gx")
                nc.vector.scalar_tensor_tensor(
                    out=gx[:], in0=dd[:, 1:20, :], scalar=2.0, in1=t2[:],
                    op0=Alu.mult, op1=Alu.add)
                gy = mid.tile([NPART, 19, WB], f32, tag="gy")
                nc.vector.tensor_tensor(
                    out=gy[:], in0=rsm[:, 2:21, :], in1=rsm[:, 0:19, :],
                    op=Alu.subtract)

                # ---- sign, abs (ACT), squares (ACT, exact), magnitude^2
                # slot reuse: sg->t2's slot, ax->tt's, ay->dd's (all dead)
                sg = mid.tile([NPART, 19, WB], f32, tag="t2")
                nc.vector.tensor_tensor(out=sg[:], in0=gx[:], in1=gy[:],
                                        op=Alu.mult)
                ax = mid.tile([NPART, 19, WB], f32, tag="tt")
                nc.scalar.activation(out=ax[:], in_=gx[:], func=Act.Abs)
                ay = mid.tile([NPART, 19, WB], f32, tag="dd")
                nc.scalar.activation(out=ay[:], in_=gy[:], func=Act.Abs)
                c0 = mid.tile([NPART, 19, WB], f32, tag="c0")
                nc.vector.scalar_tensor_tensor(
                    out=c0[:], in0=ax[:], scalar=_T1, in1=ay[:],
                    op0=Alu.mult, op1=Alu.is_ge)
                d2 = mid.tile([NPART, 19, WB], f32, tag="d2")
                nc.vector.scalar_tensor_tensor(
                    out=d2[:], in0=ax[:], scalar=_T2, in1=ay[:],
                    op0=Alu.mult, op1=Alu.is_gt)
                # squares on ACT (verified bit-exact vs IEEE mult on HW)
                nc.scalar.activation(out=gx[:], in_=gx[:], func=Act.Square)
                nc.scalar.activation(out=gy[:], in_=gy[:], func=Act.Square)
                mm = mid.tile([NPART, 19, WB], f32, tag="mm")
                nc.vector.tensor_tensor(out=mm[:], in0=gx[:], in1=gy[:],
                                        op=Alu.add)

                # ---- zero magnitude outside the image (NMS zero-padding)
                nc.vector.tensor_tensor(out=mm[:], in0=mm[:],
                                        in1=bm[:, :, 0:WB], op=Alu.mult)
                if first:
                    nc.gpsimd.memset(mm[:, :, 0:1], 0.0)      # ang col -1
                if last:
                    nc.gpsimd.memset(mm[:, :, WB - 1:WB], 0.0)  # ang col 1025

                # ---- masked magnitudes; bins processed 2,0,3,1 so ang2/
                # ang0/ang3 can share ONE slot (angA); ang1 lands in md2.
                img50 = mid.tile([NPART, 17, cw], f16, tag="img50")
                sure = mid.tile([NPART, 17, cw], f16, tag="sure")
                p16 = mid.tile([NPART, 17, cw], f16, tag="p16")
                # bins are disjoint and ang_b == M at in-bin pixels, so
                # P100_b == P50_b * [M >= 100^2]: one center-extent compare
                # replaces the whole per-bin MAX2 pass.
                big = mid.tile([NPART, 17, cw], f16, tag="big")
                nc.vector.tensor_scalar(
                    out=big[:], in0=mm[:, 1:18, 1:1 + cw], scalar1=MAX2,
                    scalar2=None, op0=Alu.is_ge)

                def nms_bin(ang, r1, c1, r2, c2, acc):
                    qt = mid.tile([NPART, 17, cw], f32, tag="rsm")
                    nc.vector.tensor_tensor(
                        out=qt[:],
                        in0=ang[:, r1:r1 + 17, c1:c1 + cw],
                        in1=ang[:, r2:r2 + 17, c2:c2 + cw],
                        op=Alu.max)
                    cen = ang[:, 1:18, 1:1 + cw]
                    dst = img50 if not acc else p16
                    nc.vector.scalar_tensor_tensor(
                        out=dst[:], in0=qt[:], scalar=MIN2, in1=cen,
                        op0=Alu.max, op1=Alu.is_le)
                    if acc:
                        # flat contiguous f16 APs -> DVE 2x mode
                        nc.vector.tensor_tensor(
                            out=_flat(img50), in0=_flat(img50),
                            in1=_flat(p16), op=Alu.add)

                md2 = mid.tile([NPART, 19, WB], f32, tag="md2")
                nc.vector.tensor_tensor(out=md2[:], in0=mm[:], in1=d2[:],
                                        op=Alu.mult)      # Md2 = M*d2
                angA = mid.tile([NPART, 19, WB], f32, tag="angA")
                nc.vector.tensor_tensor(out=angA[:], in0=mm[:], in1=md2[:],
                                        op=Alu.subtract)  # ang2 = M - Md2
                nms_bin(angA, 0, 1, 2, 1, acc=False)      # bin2: up/down
                angB = mid.tile([NPART, 19, WB], f32, tag="angA")
                nc.vector.tensor_tensor(out=angB[:], in0=md2[:], in1=c0[:],
                                        op=Alu.mult)      # ang0 = Md2*c0
                nms_bin(angB, 1, 0, 1, 2, acc=True)       # bin0: left/right
                # mdiag = Md2 - ang0 (in place; angB=ang0 still live)
                nc.vector.tensor_tensor(out=md2[:], in0=md2[:], in1=angB[:],
                                        op=Alu.subtract)
                angC = mid.tile([NPART, 19, WB], f32, tag="angA")
                nc.vector.scalar_tensor_tensor(
                    out=angC[:], in0=sg[:], scalar=0.0, in1=md2[:],
                    op0=Alu.is_gt, op1=Alu.mult)          # ang3 = (s>0)*mdiag
                nms_bin(angC, 0, 0, 2, 2, acc=True)       # bin3: main diag
                # ang1 = mdiag - ang3 (in place on md2)
                nc.vector.tensor_tensor(out=md2[:], in0=md2[:], in1=angC[:],
                                        op=Alu.subtract)
                nms_bin(md2, 0, 2, 2, 0, acc=True)        # bin1: anti diag
                # sure = img50 * [M >= 100^2]; weak = img50 - sure (all {0,1})
                nc.vector.tensor_tensor(out=_flat(sure), in0=_flat(img50),
                                        in1=_flat(big), op=Alu.mult)
                nc.vector.tensor_tensor(out=_flat(p16), in0=_flat(img50),
                                        in1=_flat(sure), op=Alu.subtract)

                # ---- scale to outputs on ACT (exact: inputs are 0/1)
                e_img = io2.tile([NPART, 17, cw], f32, tag="e_img")
                nc.scalar.activation(out=e_img[:], in_=img50[:],
                                     func=Act.Copy, scale=255.5)
                e_week = io2.tile([NPART, 17, cw], f32, tag="e_week")
                nc.scalar.activation(out=e_week[:], in_=p16[:],
                                     func=Act.Copy, scale=255.0)
                e_sure = io2.tile([NPART, 17, cw], f32, tag="e_sure")
                nc.scalar.activation(out=e_sure[:], in_=sure[:],
                                     func=Act.Copy, scale=255.0)

                for t, name in ((e_img, "o_img"), (e_week, "o_week"),
                                (e_sure, "o_sure")):
                    dst = AP(outs[name], a,
                             [[RPP * OCOLS, NPART], [OCOLS, RPP], [1, cw]])
                    nc.sync.dma_start(out=dst, in_=t[:])
                a += cw

    _split_multiwaits(nc)
    return nc


def _get_nc():
    global _NC
    if _NC is None:
        _NC = _build_nc()
    return _NC


# ------------------------------------------------------------- host helpers
def _build_q(images):
    """images: (16, 1024, 1024) f32 -> per-core padded Q (8, QROWS, QCOLS).

    Q[img_block] row r, col c = P[r-1, c-1] where P is the twice-padded
    image: P index list (both dims) = [0, 1, 0, 1, 2, ..., 1023, 1022].
    """
    idx = np.empty(1027, np.int64)
    idx[0] = 0
    idx[1] = 1
    idx[2:1026] = np.arange(1024)
    idx[1026] = 1022
    qs = np.zeros((8, QROWS, QCOLS), np.float32)
    for core in range(8):
        for k in range(NIMG):
            im = images[core * NIMG + k]
            p = im[idx][:, idx]  # (1027, 1027)
            base = k * QROWS_PER_IMG
            qs[core, base + 1: base + 1028, 1:1028] = p
    return qs


def kernel(images):
    global LAST_RESULTS
    from concourse.bass_utils import run_bass_kernel_spmd

    images = np.asarray(images, dtype=np.float32)
    assert images.shape == (16, 1024, 1024, 1), images.shape
    qs = _build_q(images[:, :, :, 0])

    bm = np.ones((NPART, 19, WBMAX), np.float32)
    for base in (0, PPI):
        bm[base, 0, :] = 0.0          # ang row -1 of each image
        bm[base + PPI - 1, 6:, :] = 0.0  # ang rows >= 1025 of each image

    nc = _get_nc()
    in_maps = [{"qin": qs[c], "bmask": bm} for c in range(8)]
    res = run_bass_kernel_spmd(nc, in_maps, list(range(8)))
    LAST_RESULTS = res

    out = []
    for name in ("o_img", "o_week", "o_sure"):
        full = np.empty((16, HO, HO, 1), np.float32)
        for c in range(8):
            r = res.results[c][name].reshape(NIMG, OROWS_PER_IMG, OCOLS)
            full[c * NIMG: c * NIMG + NIMG, :, :, 0] = r[:, :HO, :]
        out.append(full)
    return tuple(out)
